# revision 17
# baseline (speedup 1.0000x reference)
"""Trainium2 Bass kernel for Aimv2VisionEmbeddings (patch-embed GEMM + RMSNorm
+ ragged 2D sincos positional embedding), data-parallel over 8 NeuronCores.

Contract: kernel(**inputs) takes the FULL unsharded inputs and returns the
FULL [16, 4096, 1024] float32 output.

Layout/sharding: the 512 independent 128-row groups (16 batches x 32 groups)
are re-dealt across the 8 cores so that groups containing valid patches
(n < h*w, which need a positional-embedding gather) are spread evenly
(<= G_MAX per core); the rest of each core's 64 tile slots is filled with
fully-invalid groups whose pos row is the constant [0|1|0|1] block, served
from an SBUF tile built by memset - no HBM gather. This cuts the dominant
HBM stream (the 2KB/row pos gathers) by ~55% and balances it across cores,
which keeps the x-prefetch fed and the PE at its bf16 floor.

Device program per core (rows = 64 tiles x 128):
  ~22 warmup matmuls on a memset tile at t=0 bring the PE HAM clock-gate to
  K=8/8 before real data lands. GEMM on TensorE: out[r,:] accumulated in
  PSUM over 5 K-chunks (4x128 + 76, no zero-padding on the x side), per
  128-row tile. Sum-of-squares on ScalarE (Square + free-dim accum),
  rstd = rsqrt(ssq/H + eps) on VectorE via a bitcast-seed + Newton step,
  batched over pairs of tiles. Final fused (x * rstd) + pos is a single
  scalar_tensor_tensor pass straight out of PSUM, written bf16.
"""

import numpy as np
import ml_dtypes

import concourse.bass as bass
import concourse.bacc as bacc
import concourse.mybir as mybir
from concourse import tile
from concourse.bass_utils import run_bass_kernel_spmd

AF = mybir.ActivationFunctionType
ALU = mybir.AluOpType
DT = mybir.dt

B, N, D, H = 16, 4096, 588, 1024
NCORES = 8
LB = B // NCORES          # local batches per core
KCH = (128, 128, 128, 128, 76)   # contraction chunks, sum = 588
NK = len(KCH)
POS_DIM = H // 4          # 256
EPS = 1e-6
TEMP = 10000.0
QUAKE_C = 0x5F3759DF


def gather_positions(n_tiles, g_max):
    """Positions (tile slots) that perform a pos-table gather: even slots
    first, then odd - spreads gather DMA bandwidth across the kernel."""
    order = [p for p in range(n_tiles) if p % 2 == 0] + \
            [p for p in range(n_tiles) if p % 2 == 1]
    return set(order[:g_max])


def build(rows_per_b=N, g_max=28, with_bias=False, with_rmsw=False, tsz=64,
          psum_bufs=4, xt_bufs=4, work_bufs=4, pos_bufs=10, grp=2,
          warm_mms=34, out_bf16=True):
    """Build the per-core bass program. rows_per_b is shrinkable for sim."""
    rows = LB * rows_per_b
    n_tiles = rows // 128
    assert rows % 128 == 0
    out_dt = DT.bfloat16 if out_bf16 else DT.float32
    gset = gather_positions(n_tiles, g_max)

    nc = bacc.Bacc("TRN2", target_bir_lowering=False, debug=False)
    x_d = nc.declare_dram_parameter("x", [D, rows], DT.bfloat16, isOutput=False)
    w_d = nc.declare_dram_parameter("w", [NK * 128, H], DT.bfloat16, isOutput=False)
    ij_d = nc.declare_dram_parameter("ij", [rows, 1], DT.int32, isOutput=False)
    t_d = nc.declare_dram_parameter("tbl", [tsz * tsz, H], DT.bfloat16, isOutput=False)
    if with_bias:
        bias_d = nc.declare_dram_parameter("bias", [128, H], DT.float32, isOutput=False)
    if with_rmsw:
        rw_d = nc.declare_dram_parameter("rw", [128, H], DT.float32, isOutput=False)
    out_d = nc.declare_dram_parameter("out", [rows, H], out_dt, isOutput=True)

    # row blocks: small first (so the PE starts early), growing later (fewer
    # DMAs). x is SBUF-resident for the whole kernel, so blocks only set the
    # DMA granularity.
    blocks = []
    r = 0
    for size in (256, 256, 512, 1024, 1024, 2048):
        if r + size <= rows:
            blocks.append(size)
            r += size
    while r < rows:
        size = min(3072, rows - r)
        blocks.append(size)
        r += size

    kof = [sum(KCH[:k]) for k in range(NK)]  # chunk offsets in x_d

    with tile.TileContext(nc) as tc:
        with (
            tc.tile_pool(name="const", bufs=1) as cpool,
            tc.tile_pool(name="work", bufs=work_bufs) as wpool,
            tc.tile_pool(name="pos", bufs=pos_bufs) as pospool,
            tc.tile_pool(name="psum", bufs=psum_bufs, space=bass.MemorySpace.PSUM) as ppool,
        ):
            # constants built by memset (no DMA): quake constant + pos0 row
            cq = cpool.tile([128, grp], DT.int32)
            nc.vector.memset(cq[:], QUAKE_C)
            pos0 = cpool.tile([128, H], DT.bfloat16)
            nc.vector.memset(pos0[:, 0:POS_DIM], 0.0)
            nc.vector.memset(pos0[:, POS_DIM:2 * POS_DIM], 1.0)
            nc.vector.memset(pos0[:, 2 * POS_DIM:3 * POS_DIM], 0.0)
            nc.vector.memset(pos0[:, 3 * POS_DIM:4 * POS_DIM], 1.0)

            # x is fully SBUF-resident (75KB/partition at full size): every
            # x DMA is emitted upfront with zero dependencies, so no
            # buffer-recycle WARs and no HWDGE-semaphore-lane entanglement
            # between x loads and the out DMAs emitted later on this ring.
            # Block 0's x and the K-chunked weights interleave across both
            # HWDGE rings so the first tile's five chunks all land early.
            blk_tiles = []
            for blk, rbk in enumerate(blocks):
                r0 = sum(blocks[:blk])
                blk_tiles.append([
                    cpool.tile([KCH[k], rbk], DT.bfloat16, name=f"x{blk}_{k}")
                    for k in range(NK)
                ])

            ijb = cpool.tile([128, n_tiles, 1], DT.int32)
            nc.scalar.dma_start(ijb[:], ij_d[:].rearrange("(t p) c -> p t c", p=128))
            wt = cpool.tile([128, NK, H], DT.bfloat16)

            def load_block(blk):
                r0 = sum(blocks[:blk])
                for k in range(NK):
                    nc.sync.dma_start(
                        blk_tiles[blk][k][:],
                        x_d[kof[k]:kof[k] + KCH[k], r0:r0 + blocks[blk]]
                    )

            load_block(0)
            for k in range(NK):
                eng = nc.scalar if k % 2 == 0 else nc.sync
                eng.dma_start(wt[:, k, :], w_d[k * 128:(k + 1) * 128, :])
            for blk in range(1, len(blocks)):
                load_block(blk)
            if with_bias:
                biast = cpool.tile([128, H], DT.float32)
                nc.sync.dma_start(biast[:], bias_d[:])
            if with_rmsw:
                rwt = cpool.tile([128, H], DT.float32)
                nc.sync.dma_start(rwt[:], rw_d[:])

            # PE warmup: dummy matmuls on the memset pos0 tile release the
            # HAM clock-gate (K=8/8) before the first real matmul issues.
            if warm_mms:
                warm = ppool.tile([128, H], DT.float32, tag="xacc", name="warm")
                for _ in range(warm_mms):
                    nc.tensor.matmul(
                        warm[:, 0:512], pos0[:, 0:128], pos0[:, 0:512],
                        start=True, stop=True,
                    )

            pos_idx = 0  # global tile position
            for blk, rbk in enumerate(blocks):
                r0 = sum(blocks[:blk])
                tiles_per_blk = rbk // 128
                bts = blk_tiles[blk]
                last_blk = blk == len(blocks) - 1
                it = 0
                while it < tiles_per_blk:
                    g = min(grp, tiles_per_blk - it)
                    if last_blk and it >= tiles_per_blk - 2:
                        g = 1  # final two tiles solo: shortens the tail chain
                    ssqg = wpool.tile([128, grp], DT.float32, tag="ssqg")
                    xsrcs, poss = [], []
                    for gi in range(g):
                        t = it + gi
                        p = pos_idx + gi
                        xacc = ppool.tile([128, H], DT.float32, tag="xacc")
                        for half in range(2):
                            for k in range(NK):
                                nc.tensor.matmul(
                                    xacc[:, half * 512:(half + 1) * 512],
                                    bts[k][:, t * 128:(t + 1) * 128],
                                    wt[0:KCH[k], k, half * 512:(half + 1) * 512],
                                    start=(k == 0),
                                    stop=(k == NK - 1),
                                )

                        if with_bias:
                            xsrc = wpool.tile([128, H], DT.float32, tag="xb")
                            nc.vector.tensor_add(xsrc[:], xacc[:], biast[:])
                        else:
                            xsrc = xacc
                        xsrcs.append(xsrc)

                        if p in gset:
                            # gather pos rows from the sincos table (SWDGE)
                            pos = pospool.tile([128, H], DT.bfloat16, tag="pos")
                            nc.gpsimd.indirect_dma_start(
                                out=pos[:], out_offset=None, in_=t_d[:],
                                in_offset=bass.IndirectOffsetOnAxis(
                                    ap=ijb[:, p, 0:1], axis=0),
                            )
                        else:
                            pos = pos0
                        poss.append(pos)

                        # sum of squares for this tile -> ssqg[:, gi]
                        sqd = wpool.tile([128, H], DT.bfloat16, tag="sqd")
                        nc.scalar.activation(
                            sqd[:], xsrc[:], AF.Square, accum_out=ssqg[:, gi:gi + 1]
                        )

                    # rstd = rsqrt(ssq/H + eps) on DVE (bitcast seed + one
                    # Newton step; avoids a second ACT LUT table set).
                    gs = slice(0, g)
                    vq = wpool.tile([128, grp], DT.float32, tag="vq")
                    nc.vector.tensor_scalar(vq[:, gs], ssqg[:, gs], 1.0 / H, EPS, ALU.mult, ALU.add)
                    ish = wpool.tile([128, grp], DT.int32, tag="ish")
                    nc.vector.tensor_scalar(
                        ish[:, gs], vq[:, gs].bitcast(DT.int32), 1, None, ALU.arith_shift_right
                    )
                    y0 = wpool.tile([128, grp], DT.int32, tag="y0")
                    nc.vector.tensor_sub(y0[:, gs], cq[:, gs], ish[:, gs])
                    y0f = y0[:, gs].bitcast(DT.float32)
                    qa = wpool.tile([128, grp], DT.float32, tag="qa")
                    nc.vector.tensor_mul(qa[:, gs], y0f, y0f)
                    nc.vector.tensor_mul(qa[:, gs], qa[:, gs], vq[:, gs])
                    nc.vector.tensor_scalar(qa[:, gs], qa[:, gs], -0.5, 1.5, ALU.mult, ALU.add)
                    rstdg = wpool.tile([128, grp], DT.float32, tag="rstdg")
                    nc.vector.tensor_mul(rstdg[:, gs], y0f, qa[:, gs])

                    outg = wpool.tile([128, grp, H], out_dt, tag="outg")
                    for gi in range(g):
                        rs = rstdg[:, gi:gi + 1]
                        if with_rmsw:
                            xn = wpool.tile([128, H], DT.float32, tag="xn")
                            nc.vector.tensor_scalar(xn[:], xsrcs[gi][:], rs, None, ALU.mult)
                            nc.vector.tensor_mul(xn[:], xn[:], rwt[:])
                            nc.vector.tensor_add(outg[:, gi, :], xn[:], poss[gi][:])
                        else:
                            nc.vector.scalar_tensor_tensor(
                                outg[:, gi, :], xsrcs[gi][:], rs, poss[gi][:], ALU.mult, ALU.add
                            )
                    # out on the sync ring: an out DMA waits on its stts, and
                    # on the ACT ring that wait would block the next group's
                    # Square issues (strict-FIFO engine queue)
                    row0 = r0 + it * 128
                    nc.sync.dma_start(
                        out_d[row0:row0 + g * 128, :].rearrange("(a p) h -> p a h", p=128),
                        outg[:, 0:g, :],
                    )
                    it += g
                    pos_idx += g

    nc.compile()
    return nc


def plan_groups(ss, rows_per_b):
    """Deal the 128-row groups to cores: gather-needing groups round-robin
    (balanced), the rest fills the remaining slots."""
    gpb = rows_per_b // 128
    n_tiles = LB * gpb
    hw = ss[:, 0].astype(np.int64) * ss[:, 1].astype(np.int64)
    need, rest = [], []
    for b in range(B):
        for g in range(gpb):
            (need if g * 128 < hw[b] else rest).append((b, g))
    per_core_need = [need[c::NCORES] for c in range(NCORES)]
    g_max = max((len(pc) for pc in per_core_need), default=0)
    g_max = min(g_max, n_tiles)
    gset = gather_positions(n_tiles, g_max)
    rest_i = iter(rest)
    assign = []  # per core: list of (b, g) per position
    for c in range(NCORES):
        a = iter(per_core_need[c])
        slots = [None] * n_tiles
        for p in range(n_tiles):
            if p in gset:
                slots[p] = next(a, None)
        for p in range(n_tiles):
            if slots[p] is None:
                slots[p] = next(rest_i, None)
            if slots[p] is None:
                slots[p] = next(a)  # g_max > n_tiles//2 spillover case
        assign.append(slots)
    return assign, g_max


def make_inputs(hidden_states, spatial_shapes, patch_weight, patch_bias,
                rms_weight, rows_per_b=N):
    """Host-side marshalling: shard + cast + permute. Returns
    (in_maps, with_bias, with_rmsw, tsz, g_max, rows_idx)."""
    hs = np.asarray(hidden_states, dtype=np.float32)
    ss = np.asarray(spatial_shapes)
    pw = np.asarray(patch_weight, dtype=np.float32).reshape(H, D)
    pb = np.asarray(patch_bias, dtype=np.float32)
    rw = np.asarray(rms_weight, dtype=np.float32)
    with_bias = bool(np.any(pb != 0.0))
    with_rmsw = bool(np.any(rw != 1.0))

    bf16 = ml_dtypes.bfloat16
    hs2d = hs[:, :rows_per_b, :].reshape(B * rows_per_b, D)

    wp = np.zeros((NK * 128, H), dtype=bf16)
    wp[:D, :] = pw.T.astype(bf16)

    # per-row (j, i) indices, pre-masked (invalid rows -> 0), int32
    n = np.arange(rows_per_b, dtype=np.int64)[None, :]       # [1, R]
    hcol = ss[:, 0:1].astype(np.int64)
    wcol = ss[:, 1:2].astype(np.int64)
    valid = n < hcol * wcol
    jv = np.where(valid, n % wcol, 0)
    iv = np.where(valid, n // wcol, 0)
    tsz = int(max(64, jv.max() + 1, iv.max() + 1))
    ij_all = (jv * tsz + iv).reshape(B * rows_per_b).astype(np.int32)

    # universal sincos product table:
    # T[j*tsz+i] = [sin(j*om) | cos(j*om) | sin(i*om) | cos(i*om)]
    om = (1.0 / (TEMP ** (np.arange(POS_DIM, dtype=np.float64) / POS_DIM)))
    ang = np.arange(tsz, dtype=np.float64)[:, None] * om[None, :]
    sc = np.concatenate([np.sin(ang), np.cos(ang)], axis=1).astype(np.float32)  # [tsz, 512]
    tbl = np.empty((tsz, tsz, H), dtype=bf16)
    tbl[:, :, 0:512] = sc[:, None, :]
    tbl[:, :, 512:1024] = sc[None, :, :]
    tbl = np.ascontiguousarray(tbl.reshape(tsz * tsz, H))

    assign, g_max = plan_groups(ss, rows_per_b)
    ar = np.arange(128, dtype=np.int64)
    in_maps, rows_idx = [], []
    for c in range(NCORES):
        ridx = np.concatenate(
            [b * rows_per_b + g * 128 + ar for (b, g) in assign[c]]
        )
        rows_idx.append(ridx)
        xc = np.ascontiguousarray(hs2d[ridx].astype(bf16).T)  # [D, rows]
        m = {
            "x": xc,
            "w": wp,
            "ij": np.ascontiguousarray(ij_all[ridx].reshape(-1, 1)),
            "tbl": tbl,
        }
        if with_bias:
            m["bias"] = np.ascontiguousarray(np.broadcast_to(pb, (128, H)))
        if with_rmsw:
            m["rw"] = np.ascontiguousarray(np.broadcast_to(rw, (128, H)))
        in_maps.append(m)
    return in_maps, with_bias, with_rmsw, tsz, g_max, rows_idx


_BUILD_CACHE = {}


def kernel(hidden_states, spatial_shapes, patch_weight, patch_bias, rms_weight,
           _trace=False):
    in_maps, with_bias, with_rmsw, tsz, g_max, rows_idx = make_inputs(
        hidden_states, spatial_shapes, patch_weight, patch_bias, rms_weight
    )
    key = (with_bias, with_rmsw, tsz, g_max)
    if key not in _BUILD_CACHE:
        _BUILD_CACHE[key] = build(with_bias=with_bias, with_rmsw=with_rmsw,
                                  tsz=tsz, g_max=g_max)
    nc = _BUILD_CACHE[key]
    if not getattr(kernel, "_warm", False):
        run_bass_kernel_spmd(nc, in_maps, list(range(NCORES)))
        kernel._warm = True
    res = run_bass_kernel_spmd(nc, in_maps, list(range(NCORES)), trace=_trace)
    out = np.empty((B * N, H), dtype=np.float32)
    for c in range(NCORES):
        out[rows_idx[c]] = res.results[c]["out"].astype(np.float32)
    out = out.reshape(B, N, H)
    if _trace:
        kernel.last_results = res
    return out
